# revision 4
# baseline (speedup 1.0000x reference)
"""v3 Trainium2 Bass kernel for nn_DeformAtten1D.

Changes vs v2 baseline (all aimed at PE/DVE time):
- offset conv restructured: taps-as-partitions z20 matmuls (8 x 260-row
  matmuls per batch vs 160 x 512-row) + Pool shifted-adds -> acc rows
- offsets kept in row layout [g, l]; tanh/scale/crow fused on ACT/Pool
- sampler S^T built directly in [r, l'] layout via partition_broadcast +
  const D tables (drops 2 PE transposes + 2 DVE copies per (g,s))
- qT built with dma_start_transpose (xbar) instead of 128 PE transposes
  + 128 DVE copies per batch
- xT loaded in (kt, nb) chunks so q-proj starts ~20us earlier
- ao/xs PSUM evacuations moved partly to ACT
"""
import sys
for _p in ('/opt/trn_rl_repo', '/root/.axon_site/_ro/trn_rl_repo'):
    if _p not in sys.path:
        sys.path.insert(0, _p)

import numpy as np
import ml_dtypes

import concourse.bass as bass
import concourse.bacc as bacc
import concourse.mybir as mybir
import concourse.tile as tile
from concourse.masks import make_identity
from concourse.library_config import mlp

F32 = mybir.dt.float32
BF16 = mybir.dt.bfloat16
AF = mybir.ActivationFunctionType
OP = mybir.AluOpType
BF = ml_dtypes.bfloat16


class Cfg:
    def set_mm_gather(self, on=True):
        self.mm_gather = on
        self.GO, self.GR = 9, 32

    def __init__(self, B_SH, L, C, H, G, K=5):
        self.B_SH, self.L, self.C, self.H, self.G, self.K = B_SH, L, C, H, G, K
        self.GC = C // G
        self.DH = C // H
        assert self.DH == 64
        self.PAD = K // 2
        self.L4 = L + 2 * self.PAD
        self.sn = L / (self.L4 - 1)
        self.KT = C // 128
        self.NS = L // 128
        self.SLG = self.GC // 128
        self.NB = L // 512
        self.NC2 = L // 256          # 256-wide conv blocks
        self.NO = C // 512
        self.MH = self.KT // self.NO
        self.GO, self.GR = 9, 32
        self.P2 = H // 2
        assert self.P2 == self.KT
        assert L % 512 == 0 and C % 512 == 0 and self.GC % 128 == 0


def declare(nc, cfg):
    c = cfg
    t = {}
    t['xgbf'] = nc.dram_tensor("xgbf", [c.B_SH * (c.L + c.GR), c.C], BF16, kind="ExternalInput")
    t['xtbf'] = nc.dram_tensor("xtbf", [c.B_SH * c.C, c.L], BF16, kind="ExternalInput")
    t['rtab'] = nc.dram_tensor("rtab", [c.C, c.L], BF16, kind="ExternalInput")
    t['wpr20'] = nc.dram_tensor("wpr20", [128, c.KT, 4 * c.K], BF16, kind="ExternalInput")
    for nm in ('wqT', 'wkT', 'wvT', 'woutT'):
        t[nm] = nc.dram_tensor(nm, [c.C, c.C], BF16, kind="ExternalInput")
    t['bq_col'] = nc.dram_tensor("bq_col", [c.C, 1], F32, kind="ExternalInput")
    t['bk_row'] = nc.dram_tensor("bk_row", [1, c.C], BF16, kind="ExternalInput")
    t['bo_row'] = nc.dram_tensor("bo_row", [1, c.C], BF16, kind="ExternalInput")
    t['bcompc'] = nc.dram_tensor("bcompc", [1, 1], F32, kind="ExternalInput")
    t['w2b1c'] = nc.dram_tensor("w2b1c", [1, 1], F32, kind="ExternalInput")
    t['dtab'] = nc.dram_tensor("dtab", [160, 128], F32, kind="ExternalInput")
    t['out2d'] = nc.dram_tensor("out2d", [c.B_SH * c.L, c.C], BF16, kind="ExternalOutput")
    if getattr(c, 'debug', False):
        t['d_gq'] = nc.dram_tensor("d_gq", [128, c.KT, c.L + 4], BF16, kind="ExternalOutput")
        t['d_acc2'] = nc.dram_tensor("d_acc2", [97, c.L], F32, kind="ExternalOutput")
        t['d_xs'] = nc.dram_tensor("d_xs", [128, c.KT, c.L], BF16, kind="ExternalOutput")
        t['d_qsl'] = nc.dram_tensor("d_qsl", [128, c.NS, 128], BF16, kind="ExternalOutput")
        t['d_ktt'] = nc.dram_tensor("d_ktt", [128, c.NS, c.C], BF16, kind="ExternalOutput")
        t['d_v'] = nc.dram_tensor("d_v", [128, c.KT, c.L], BF16, kind="ExternalOutput")
    return t


def build(tc, t, cfg, ctx):
    c = cfg
    nc = tc.nc
    L, C, KT, NS, NB, NO, GC, SLG, G, K, MH = (c.L, c.C, c.KT, c.NS, c.NB, c.NO,
                                               c.GC, c.SLG, c.G, c.K, c.MH)
    NC2 = c.NC2
    scale = C ** -0.5

    nc.gpsimd.load_library(mlp)

    konst = ctx.enter_context(tc.tile_pool(name="konst", bufs=1))
    big = ctx.enter_context(tc.tile_pool(name="big", bufs=1))
    wp = ctx.enter_context(tc.tile_pool(name="wp", bufs=3))
    sm = ctx.enter_context(tc.tile_pool(name="sm", bufs=3))
    rp = ctx.enter_context(tc.tile_pool(name="rp", bufs=2))
    yp = ctx.enter_context(tc.tile_pool(name="yp", bufs=3))
    qp = ctx.enter_context(tc.tile_pool(name="qp", bufs=1))
    convp = ctx.enter_context(tc.tile_pool(name="convp", bufs=1))
    ztp = ctx.enter_context(tc.tile_pool(name="ztp", bufs=2))
    sgp = ctx.enter_context(tc.tile_pool(name="sgp", bufs=2))
    dscr = ctx.enter_context(tc.tile_pool(name="dscr", bufs=2, space="DRAM"))
    psmm = ctx.enter_context(tc.tile_pool(name="psmm", bufs=3, space="PSUM"))
    pssc = ctx.enter_context(tc.tile_pool(name="pssc", bufs=2, space="PSUM"))
    psz = ctx.enter_context(tc.tile_pool(name="psz", bufs=2, space="PSUM"))

    # ---- constants ----
    wpr20_sb = konst.tile([128, KT, 4 * K], BF16, tag="wpr20")
    nc.sync.dma_start(out=wpr20_sb[:], in_=t['wpr20'].ap())
    bq_col_sb = konst.tile([128, KT], F32, tag="bqc")
    nc.sync.dma_start(out=bq_col_sb[:], in_=bass.AP(
        tensor=t['bq_col'].ap().tensor, offset=0, ap=[[1, 128], [128, KT]]))
    bcomp_sb = konst.tile([128, 1], F32, tag="bco")
    nc.sync.dma_start(out=bcomp_sb[:], in_=bass.AP(
        tensor=t['bcompc'].ap().tensor, offset=0, ap=[[0, 128], [1, 1]]))
    w2b1_sb = konst.tile([128, 1], F32, tag="w2b1")
    nc.sync.dma_start(out=w2b1_sb[:], in_=bass.AP(
        tensor=t['w2b1c'].ap().tensor, offset=0, ap=[[0, 128], [1, 1]]))
    bkr_sb = konst.tile([1, C], BF16, tag="bkr")
    nc.sync.dma_start(out=bkr_sb[:], in_=t['bk_row'].ap())
    bor_sb = konst.tile([1, C], BF16, tag="bor")
    nc.sync.dma_start(out=bor_sb[:], in_=t['bo_row'].ap())
    bkr_bc = konst.tile([128, C], BF16, tag="bkrb")
    nc.gpsimd.partition_broadcast(bkr_bc[:], bkr_sb[:])
    bor_bc = konst.tile([128, C], BF16, tag="borb")
    nc.gpsimd.partition_broadcast(bor_bc[:], bor_sb[:])
    onef_col = konst.tile([128, 1], F32, tag="onef")
    nc.vector.memset(onef_col[:], 1.0)
    d0_sb = konst.tile([128, 128], F32, tag="d0")
    nc.sync.dma_start(out=d0_sb[:], in_=t['dtab'].ap()[0:128, :])
    d1_sb = konst.tile([32, 128], F32, tag="d1")
    nc.sync.dma_start(out=d1_sb[:], in_=t['dtab'].ap()[128:160, :])
    ident = konst.tile([128, 128], F32, tag="ident")
    make_identity(nc, ident[:])

    xg = t['xgbf'].ap()
    LG = L + c.GR

    def load_whalf(wname, hi):
        wh = wp.tile([128, KT, 512], BF16, tag="wblk")
        nc.sync.dma_start(out=wh[:], in_=bass.AP(
            tensor=t[wname].ap().tensor, offset=512 * hi,
            ap=[[C, 128], [128 * C, KT], [1, 512]]))
        return wh

    for b in range(c.B_SH):
        # ================= phase A: xT load (chunked), q-pass =================
        wq_hs = [load_whalf('wqT', hi) for hi in range(NO)]
        xT = big.tile([128, KT, L], BF16, tag="bigX")
        for nb in range(NB):
            for kt in range(KT):
                nc.sync.dma_start(
                    out=xT[:, kt, 512 * nb:512 * (nb + 1)],
                    in_=t['xtbf'].ap()[b * C + 128 * kt: b * C + 128 * (kt + 1),
                                       512 * nb:512 * (nb + 1)])

        gq = big.tile([128, KT, L + 4], BF16, tag="bigGV")
        nc.gpsimd.memset(gq[:, :, 0:4], 0.0)
        for hi in range(NO):
            wq_h = wq_hs[hi]
            for n in range(NB):
                for mm_ in range(MH):
                    m = hi * MH + mm_
                    ps = psmm.tile([128, 512], F32, tag="mm", space="PSUM")
                    for kt in range(KT):
                        nc.tensor.matmul(ps[:], lhsT=wq_h[:, kt, 128 * mm_:128 * (mm_ + 1)],
                                         rhs=xT[:, kt, 512 * n:512 * (n + 1)],
                                         start=(kt == 0), stop=(kt == KT - 1))
                    nc.scalar.activation(out=gq[:, m, 4 + 512 * n:4 + 512 * (n + 1)], in_=ps[:],
                                         func=AF.Identity, bias=bq_col_sb[:, m:m + 1], scale=1.0)

        # ====== phase B0: offset conv, taps-as-partitions ======
        # z20[4*tt+g, j] = sum_c wpr20[c, kt, 4tt+g] * gq[c, kt, 256n'+j]
        # group rows live at partitions {0,32,64,96} (engine base-partition rule)
        # group rows contiguous 0..3 for engine ops (walrus: contiguous
        # partitions only, base in {0,32,64,96})
        acc = convp.tile([4, L], F32, tag="acc")
        accd = dscr.tile([4, L], F32, tag="accd")
        for n2 in range(NC2):
            z20 = psz.tile([128, 512], F32, tag="z20", space="PSUM")
            for kt in range(KT):
                nc.tensor.matmul(z20[0:4 * K, 0:260],
                                 lhsT=wpr20_sb[:, kt, :],
                                 rhs=gq[:, kt, 256 * n2:256 * n2 + 260],
                                 start=(kt == 0), stop=(kt == KT - 1))
            zsb = ztp.tile([4 * K, 260], F32, tag="zsb")
            nc.scalar.copy(out=zsb[:], in_=z20[0:4 * K, 0:260])
            # DMA-rearrange taps into free-dim slabs on partitions 0..3
            ztap = ztp.tile([4, K, 264], F32, tag="ztap")
            for tt in range(K):
                nc.sync.dma_start(out=ztap[0:4, tt, 0:260],
                                  in_=zsb[4 * tt:4 * (tt + 1), :])
            ach = acc[0:4, 256 * n2:256 * (n2 + 1)]
            nc.gpsimd.tensor_tensor(out=ach, in0=ztap[0:4, 0, 0:256],
                                    in1=ztap[0:4, 1, 1:257], op=OP.add)
            for tt in range(2, K):
                nc.gpsimd.tensor_tensor(out=ach, in0=ach,
                                        in1=ztap[0:4, tt, tt:tt + 256],
                                        op=OP.add)
        # conv2 zero-pad boundary: off pre-tanh at l' in {0,1} is -w2b1
        nc.vector.tensor_scalar(out=acc[0:4, 0:2], in0=acc[0:4, 0:2],
                                scalar1=0.0, scalar2=None, op0=OP.mult)
        nc.vector.tensor_scalar(out=acc[0:4, 0:2], in0=acc[0:4, 0:2],
                                scalar1=w2b1_sb[0:4, :], scalar2=None,
                                op0=OP.subtract)
        # acc <- tanh(acc + bcomp) per 512-chunk, stage rows to DRAM for the
        # broadcast-read DMAs (HW partition_broadcast can't source partition!=0)
        for n4 in range(L // 512):
            nc.scalar.activation(out=acc[0:4, 512 * n4:512 * (n4 + 1)],
                                 in_=acc[0:4, 512 * n4:512 * (n4 + 1)],
                                 func=AF.Tanh, bias=bcomp_sb[0:4, 0:1], scale=1.0)
            nc.sync.dma_start(out=accd[0:4, 512 * n4:512 * (n4 + 1)],
                              in_=acc[0:4, 512 * n4:512 * (n4 + 1)])
        accd_ap = accd[:]

        # ====== phase B2: sampler: S^T built directly, banded matmul ======
        xs = big.tile([128, KT, L], BF16, tag="bigX")
        x_lc = big.tile([128, NS + 1, C], BF16, tag="bigKA")
        for s in range(NS):
            nc.sync.dma_start(out=x_lc[:, s, :],
                              in_=xg[b * LG + 128 * s: b * LG + 128 * (s + 1), :])
        nc.sync.dma_start(out=x_lc[0:32, NS, :],
                          in_=xg[b * LG + 128 * NS: b * LG + 128 * NS + 32, :])
        wk_hs = [load_whalf('wkT', hi) for hi in range(NO)]
        qsls = []
        kTt = big.tile([128, NS, C], BF16, tag="bigKA2")
        for s in range(NS):
            for g in range(G):
                csI = float(128.0 * s * (1.0 - c.sn) / (5.0 * c.sn))
                bc = sgp.tile([128, 128], F32, tag="bc")
                nc.sync.dma_start(out=bc[:], in_=bass.AP(
                    tensor=accd_ap.tensor,
                    offset=accd_ap.offset + g * L + 128 * s,
                    ap=[[0, 128], [1, 128]]))
                d1t = sgp.tile([128, 128], F32, tag="d1t")
                nc.vector.scalar_tensor_tensor(out=d1t[:], in0=d0_sb[:], scalar=csI,
                                               in1=bc[:], op0=OP.add, op1=OP.subtract)
                d2t = sgp.tile([32, 128], F32, tag="d2t")
                nc.vector.scalar_tensor_tensor(out=d2t[:], in0=d1_sb[:], scalar=csI,
                                               in1=bc[0:32, :], op0=OP.add, op1=OP.subtract)
                nc.vector.scalar_tensor_tensor(out=d1t[:], in0=d1t[:], scalar=-1.0,
                                               in1=d1t[:], op0=OP.mult, op1=OP.max)
                nc.vector.scalar_tensor_tensor(out=d2t[:], in0=d2t[:], scalar=-1.0,
                                               in1=d2t[:], op0=OP.mult, op1=OP.max)
                sg1 = sgp.tile([128, 128], BF16, tag="sg1")
                nc.scalar.activation(out=sg1[:], in_=d1t[:], func=AF.Relu,
                                     bias=onef_col[0:128, 0:1],
                                     scale=float(-5.0 * c.sn))
                sg2 = sgp.tile([32, 128], BF16, tag="sg2")
                nc.scalar.activation(out=sg2[:], in_=d2t[:], func=AF.Relu,
                                     bias=onef_col[0:32, 0:1],
                                     scale=float(-5.0 * c.sn))
                ps = psmm.tile([128, 512], F32, tag="mm", space="PSUM")
                for j in range(SLG):
                    nc.tensor.matmul(ps[:, 128 * j:128 * (j + 1)],
                                     lhsT=x_lc[:, s, GC * g + 128 * j:GC * g + 128 * (j + 1)],
                                     rhs=sg1[:], start=True, stop=False)
                    nc.tensor.matmul(ps[:, 128 * j:128 * (j + 1)],
                                     lhsT=x_lc[0:32, s + 1, GC * g + 128 * j:GC * g + 128 * (j + 1)],
                                     rhs=sg2[:], start=False, stop=True)
                if g % 2 == 0:
                    nc.vector.tensor_copy(out=xs[:, SLG * g:SLG * (g + 1), 128 * s:128 * (s + 1)],
                                          in_=ps[:, 0:256])
                else:
                    nc.scalar.copy(out=xs[:, SLG * g:SLG * (g + 1), 128 * s:128 * (s + 1)],
                                   in_=ps[:, 0:256])
            # k-projection for this column block (keeps PE fed over the S build)
            for hi in range(NO):
                ps = psmm.tile([128, 512], F32, tag="mm", space="PSUM")
                for kt in range(KT):
                    nc.tensor.matmul(ps[:], lhsT=xs[:, kt, 128 * s:128 * (s + 1)],
                                     rhs=wk_hs[hi][:, kt, :],
                                     start=(kt == 0), stop=(kt == KT - 1))
                nc.vector.tensor_tensor(out=kTt[:, s, 512 * hi:512 * (hi + 1)],
                                        in0=ps[:], in1=bkr_bc[:, 512 * hi:512 * (hi + 1)],
                                        op=OP.add)
            if s == 1:
                # qT xbar transposes: issued once x_lc loads have drained
                for pr in range(c.P2):
                    qsl = qp.tile([128, NS, 128], BF16, tag=f"qsl{pr}")
                    nc.sync.dma_start_transpose(qsl[:], gq[:, pr, 4:4 + L])
                    qsls.append(qsl)

        if getattr(c, 'debug', False) and b == 0:
            nc.sync.dma_start(out=t['d_qsl'].ap(), in_=qsls[0][:])
            nc.sync.dma_start(out=t['d_ktt'].ap(), in_=kTt[:])
        # scores + softmax + transposed block-diag attn (pairs of heads)
        attnTs = []
        for pr in range(c.P2):
            ps_sc = pssc.tile([128, 128], F32, tag="sc", space="PSUM")
            qsl = qsls[pr]
            for lt in range(NS):
                nc.tensor.matmul(ps_sc[:], lhsT=qsl[:, lt, :],
                                 rhs=kTt[:, lt, 128 * pr:128 * (pr + 1)],
                                 start=(lt == 0), stop=(lt == NS - 1))
            rmax = sm.tile([128, 1], F32, tag="rmax")
            nc.vector.reduce_max(out=rmax[0:64, :], in_=ps_sc[0:64, 0:64],
                                 axis=mybir.AxisListType.X)
            nc.vector.reduce_max(out=rmax[64:128, :], in_=ps_sc[64:128, 64:128],
                                 axis=mybir.AxisListType.X)
            nb_ = sm.tile([128, 1], F32, tag="nb")
            nc.vector.tensor_scalar(out=nb_[:], in0=rmax[:], scalar1=-scale, scalar2=None, op0=OP.mult)
            expt = sm.tile([128, 64], F32, tag="expt")
            nc.scalar.activation(out=expt[0:64, :], in_=ps_sc[0:64, 0:64], func=AF.Exp,
                                 bias=nb_[0:64, :], scale=scale)
            nc.scalar.activation(out=expt[64:128, :], in_=ps_sc[64:128, 64:128], func=AF.Exp,
                                 bias=nb_[64:128, :], scale=scale)
            rsum = sm.tile([128, 1], F32, tag="rsum")
            nc.vector.reduce_sum(out=rsum[:], in_=expt[:], axis=mybir.AxisListType.X)
            rinv = sm.tile([128, 1], F32, tag="rinv")
            nc.vector.reciprocal(out=rinv[:], in_=rsum[:])
            ablk = sm.tile([128, 128], F32, tag="ablk")
            nc.gpsimd.memset(ablk[:], 0.0)
            nc.vector.tensor_scalar(out=ablk[0:64, 0:64], in0=expt[0:64, :],
                                    scalar1=rinv[0:64, :], scalar2=None, op0=OP.mult)
            nc.vector.tensor_scalar(out=ablk[64:128, 64:128], in0=expt[64:128, :],
                                    scalar1=rinv[64:128, :], scalar2=None, op0=OP.mult)
            trp = psmm.tile([128, 512], F32, tag="mm", space="PSUM")
            nc.tensor.transpose(trp[:, 0:128], ablk[:], ident[:])
            aT = qp.tile([128, 128], BF16, tag=f"aT{pr}")
            nc.vector.tensor_copy(out=aT[:], in_=trp[:, 0:128])
            attnTs.append(aT)

        # ============== phase D: v-pass + attn@v ==============
        v = big.tile([128, KT, L], BF16, tag="bigGV")
        for hi in range(NO):
            wv_h = load_whalf('wvT', hi)
            for mm_ in range(MH):
                m = hi * MH + mm_
                for n in range(NB):
                    ps = psmm.tile([128, 512], F32, tag="mm", space="PSUM")
                    for kt in range(KT):
                        nc.tensor.matmul(ps[:], lhsT=wv_h[:, kt, 128 * mm_:128 * (mm_ + 1)],
                                         rhs=xs[:, kt, 512 * n:512 * (n + 1)],
                                         start=(kt == 0), stop=(kt == KT - 1))
                    rt = rp.tile([128, 512], BF16, tag="rt")
                    nc.sync.dma_start(out=rt[:], in_=t['rtab'].ap()[128 * m:128 * (m + 1),
                                                                    512 * n:512 * (n + 1)])
                    nc.vector.tensor_tensor(out=v[:, m, 512 * n:512 * (n + 1)],
                                            in0=ps[:], in1=rt[:], op=OP.add)

        # attn @ v -> ao^T  [o, l]
        ao = big.tile([128, KT, L], BF16, tag="bigKA")
        for pr in range(c.P2):
            for n in range(NB):
                ps = psmm.tile([128, 512], F32, tag="mm", space="PSUM")
                nc.tensor.matmul(ps[:], lhsT=attnTs[pr][:],
                                 rhs=v[:, pr, 512 * n:512 * (n + 1)],
                                 start=True, stop=True)
                nc.scalar.copy(out=ao[:, pr, 512 * n:512 * (n + 1)], in_=ps[:])

        # ====== phase E: x-stationary out projection -> out2d [l, o] contiguous ======
        out_ap = t['out2d'].ap()
        for hi in range(NO):
            wo_h = load_whalf('woutT', hi)
            for lt in range(NS):
                ps = psmm.tile([128, 512], F32, tag="mm", space="PSUM")
                for kt in range(KT):
                    nc.tensor.matmul(ps[:], lhsT=ao[:, kt, 128 * lt:128 * (lt + 1)],
                                     rhs=wo_h[:, kt, :],
                                     start=(kt == 0), stop=(kt == KT - 1))
                yt = yp.tile([128, 512], BF16, tag="yt")
                nc.vector.tensor_tensor(out=yt[:], in0=ps[:],
                                        in1=bor_bc[:, 512 * hi:512 * (hi + 1)],
                                        op=OP.add)
                nc.sync.dma_start(out=bass.AP(
                    tensor=out_ap.tensor, offset=(b * L + 128 * lt) * C + 512 * hi,
                    ap=[[C, 128], [1, 512]]), in_=yt[:])


def make_nc(cfg):
    nc = bacc.Bacc("TRN2", target_bir_lowering=False, debug=False)
    t = declare(nc, cfg)
    from contextlib import ExitStack
    with tile.TileContext(nc) as tc:
        with ExitStack() as ctx:
            build(tc, t, cfg, ctx)
    nc.compile()
    return nc


def host_prep_shared(inputs, cfg):
    c = cfg
    Wq, Wk, Wv, Wout = inputs['Wq'], inputs['Wk'], inputs['Wv'], inputs['Wout']
    w2 = np.asarray(inputs['Woff2'][0, :, 0], np.float32)          # [GC]
    W1 = np.asarray(inputs['Woff1'], np.float32)                   # [GC, GC, K]
    wpr = np.einsum('c,cik->ik', w2, W1)                           # [GC, K]
    w2b1 = float(np.dot(w2, np.asarray(inputs['boff1'], np.float32)))
    bcomp = w2b1 + float(np.asarray(inputs['boff2']).reshape(-1)[0])
    # block-sparse taps-as-partitions composite weight:
    # wpr20[cp, kt, 4*tt+g] = wpr[128*(kt-2g)+cp, tt] if kt//2 == g else 0
    wpr20 = np.zeros((128, c.KT, 4 * c.K), np.float32)
    for kt in range(c.KT):
        g = kt // 2
        kt2 = kt - 2 * g
        for tt in range(c.K):
            wpr20[:, kt, 4 * tt + g] = wpr[128 * kt2:128 * (kt2 + 1), tt]
    # D table in hat units: d[r, j] = (r - j*sn - 8.5)/(5*sn)
    r = np.arange(160, dtype=np.float32)[:, None]
    j = np.arange(128, dtype=np.float32)[None, :]
    dtab = ((r - j * c.sn - 8.5) / (5.0 * c.sn)).astype(np.float32)
    dtab[128 + 32:] = 1e6   # unused
    sh = {
        'wqT': np.ascontiguousarray(Wq.T).astype(BF),
        'wkT': np.ascontiguousarray(Wk.T).astype(BF),
        'wvT': np.ascontiguousarray(Wv.T).astype(BF),
        'woutT': np.ascontiguousarray(Wout.T).astype(BF),
        'wpr20': wpr20.astype(BF),
        'bq_col': inputs['bq'][:, None].astype(np.float32),
        'bk_row': inputs['bk'][None, :].astype(BF),
        'bo_row': inputs['bout'][None, :].astype(BF),
        'bcompc': np.array([[bcomp]], np.float32),
        'w2b1c': np.array([[w2b1]], np.float32),
        'rtab': (inputs['bv'][:, None] + inputs['rpb_table'][0]).astype(BF),
        'dtab': dtab,
    }
    return sh


def host_prep_core(x_shard, cfg):
    c = cfg
    xgp = np.zeros((c.B_SH, c.L + c.GR, c.C), np.float32)
    xgp[:, c.GO:c.GO + c.L] = x_shard
    xt = np.swapaxes(np.asarray(x_shard, np.float32), 1, 2)        # [B_SH, C, L]
    return {
        'xgbf': xgp.reshape(c.B_SH * (c.L + c.GR), c.C).astype(BF),
        'xtbf': np.ascontiguousarray(xt).reshape(c.B_SH * c.C, c.L).astype(BF),
    }


# ----------------------------------------------------------------------------
# Public entry point
# ----------------------------------------------------------------------------
_N_CORES = 8
_B, _L, _C, _H, _G, _K = 16, 2048, 1024, 16, 4, 5
_CACHE = {}


def _get_nc(cfg):
    if 'nc' not in _CACHE:
        _CACHE['nc'] = make_nc(cfg)
    return _CACHE['nc']


def kernel(**inputs):
    inputs = {k: np.asarray(v) for k, v in inputs.items()}
    cfg = Cfg(B_SH=_B // _N_CORES, L=_L, C=_C, H=_H, G=_G, K=_K)
    cfg.set_mm_gather(True)
    nc = _get_nc(cfg)
    sh = host_prep_shared(inputs, cfg)
    in_maps = [
        {**sh, **host_prep_core(inputs['x'][c * cfg.B_SH:(c + 1) * cfg.B_SH], cfg)}
        for c in range(_N_CORES)
    ]
    from concourse.bass_utils import run_bass_kernel_spmd
    res = run_bass_kernel_spmd(nc, in_maps, core_ids=list(range(_N_CORES)))
    out = np.concatenate(
        [res.results[c]["out2d"].reshape(cfg.B_SH, _L, _C) for c in range(_N_CORES)],
        axis=0)
    return out.astype(np.float32)
